# revision 3
# baseline (speedup 1.0000x reference)
"""Multi-head attention (12 heads, dh=64) + output projection on 8 TRN2 NeuronCores.

Data-parallel: one batch element per core, no collectives.

Same math as the original baseline (fp16 QK path for softmax accuracy,
aug-row trick so the transposed-S matmul computes k.q - m[q] directly,
ones-column in V so P@V also emits rowsum(P), K=1 matmul broadcast for the
1/s normalization), restructured for engine overlap:
  - generator-interleaved emission (`_drive` round-robins per-phase
    generators): head h's ST'/exp/PV stream is woven with head h+1's pass1
    (S tiles + DVE rowmax) and head h+3's qk projection, so the PE/ACT/DVE
    FIFOs always hold ready work.
  - [128,512] single-PSUM-bank granularity everywhere with 2-slot rotation
    per stream; PV lags ST'/exp by LOOKAHEAD steps so the first PV matmuls
    of a head (which wait on the previous head's psOT drain) never starve
    the PE FIFO.
  - norm chain: Ln reads s directly from PSUM row 64; rs produced in bf16-
    compatible fp16 by the ACT exp; psOT is two [65,512] single-bank halves.
  - PSUM: sp (pass1 S halves x2) + stp (ST' stream x2, also psRS/V/fc) +
    pp (proj psqk) + op (psOT halves) = exactly 8 banks.
  - merged prologue (V-proj || proj0 || pass1(0), interleaved input DMAs)
    and 4-slot fc tail.

build_graph(repeat=N) unrolls the whole computation N times inside one NEFF
for drift-free throughput benchmarking (see hwbench.py); kernel() uses N=1.

Sim (TimelineSim): 244.5 us. Measured sustained per-iteration on HW:
~315-348 us vs the original baseline's ~353 us (same-process R-slope).
"""

import os
import sys
from contextlib import ExitStack

import numpy as np

for _p in ("/opt/trn_rl_repo",):
    if _p not in sys.path and os.path.isdir(_p):
        sys.path.insert(0, _p)

import ml_dtypes  # noqa: E402

import concourse.bass as bass  # noqa: E402
import concourse.tile as tile  # noqa: E402
from concourse import mybir  # noqa: E402
from concourse.bass import ds, ts  # noqa: E402
from concourse.bass_utils import run_bass_kernel_spmd  # noqa: E402
from concourse.masks import make_identity  # noqa: E402

P = 128
NT = 1024   # tokens per core (batch element)
D = 768     # model dim
DC = D // P  # 6 contraction chunks
H = 12      # heads
DH = 64     # head dim
QT = NT // P  # 8 q tiles
KT = NT // P  # 8 k tiles
E3 = 3 * D  # 2304

F32 = mybir.dt.float32
F16 = mybir.dt.float16
BF16 = mybir.dt.bfloat16

N_CORES = 8


def _split_sync_waits(nc, max_waits=1):
    """Walrus codegen allows only a limited number of semaphore waits per
    instruction. Move extra waits onto same-engine NoOps inserted immediately
    before the offending instruction."""
    from concourse import mybir as mb
    for f in nc.m.functions:
        for b in f.blocks:
            out = []
            changed = False
            for inst in b.instructions:
                si = inst.sync_info
                waits = list(si.on_wait) if (si is not None and si.on_wait) else []
                eng = getattr(inst, "engine", None)
                if (type(inst).__name__ == "InstISA"
                        and getattr(inst, "op_name", None) == "EVENT_SEMAPHORE_RANGE_CLEAR"):
                    lo, hi = inst.instr[13], inst.instr[14]
                    for sid in range(lo, hi + 1):
                        out.append(mb.InstEventSemaphore(
                            name=nc.get_next_instruction_name(),
                            sync_info=mb.SyncInfo(on_wait=[], on_update=[
                                mb.SyncUpdate(sync_type="semaphore", id=sid,
                                              ant_name=f"semclr_{sid}",
                                              update_mode="sem-wr-imm",
                                              update_value=0, update_reg=None)]),
                            engine=eng,
                        ))
                    changed = True
                    continue
                lim = max_waits
                if len(waits) > lim and eng is not None:
                    for w in waits[:-lim]:
                        nop = mb.InstEventSemaphore(
                            name=nc.get_next_instruction_name(),
                            sync_info=mb.SyncInfo(on_wait=[w], on_update=[]),
                            engine=eng,
                        )
                        out.append(nop)
                    inst.sync_info = mb.SyncInfo(
                        on_wait=waits[-lim:],
                        on_update=list(si.on_update) if si.on_update else [],
                    )
                    changed = True
                out.append(inst)
            if changed:
                b.instructions = out


def _delay(gen, n):
    """Emit n empty quanta before starting gen."""
    for _ in range(n):
        yield
    yield from gen


def _every(gen, n):
    """Pace a generator: emit one quantum every n driver rounds."""
    for x in gen:
        yield
        for _ in range(n - 1):
            yield


def _drive(*gens):
    """Round-robin the generators until all are exhausted. Each yield is one
    emission quantum; the weave keeps every engine's queue fed."""
    live = [g for g in gens if g is not None]
    while live:
        nxt = []
        for g in live:
            try:
                next(g)
                nxt.append(g)
            except StopIteration:
                pass
        live = nxt


def build_graph(repeat=1):
    nc = bass.Bass()
    imgT = nc.declare_dram_parameter("imgT", [D, NT], F16, isOutput=False)
    WqkvT = nc.declare_dram_parameter("WqkvT", [D, E3], F16, isOutput=False)
    WfcT = nc.declare_dram_parameter("WfcT", [D, D], BF16, isOutput=False)
    b_fc = nc.declare_dram_parameter("b_fc", [D], F32, isOutput=False)
    out = nc.declare_dram_parameter("out", [NT, D], F32, isOutput=True)

    with tile.TileContext(nc) as tc, ExitStack() as ctx:
        const = ctx.enter_context(tc.tile_pool(name="const", bufs=1))
        aug = ctx.enter_context(tc.tile_pool(name="aug", bufs=8))
        ptp = ctx.enter_context(tc.tile_pool(name="ptp", bufs=8))
        small = ctx.enter_context(tc.tile_pool(name="small", bufs=4))
        outp = ctx.enter_context(tc.tile_pool(name="outp", bufs=3))
        # PSUM pools, 8 banks total:
        #  sp: pass1 S half-tiles [128,512] x2 (2 banks); psT borrows
        #  stp: ST' stream half-tiles [128,512] x2 (2 banks); psRS, V-proj
        #       and fc halves borrow
        #  pp: proj psqk [128,1024] (2 banks)
        #  op: psOT halves [65,512] x2 (2 banks)
        sp = ctx.enter_context(tc.tile_pool(name="sp", bufs=2, space="PSUM"))
        stp = ctx.enter_context(tc.tile_pool(name="stp", bufs=2, space="PSUM"))
        pp = ctx.enter_context(tc.tile_pool(name="pp", bufs=1, space="PSUM"))
        op = ctx.enter_context(tc.tile_pool(name="op", bufs=2, space="PSUM"))

        # ---- input loads (split per contraction chunk so compute starts early) ----
        img_sb = []
        wq_sb = []
        wf_sb = []
        for c in range(DC):
            t = const.tile([P, NT], F16, tag=f"img{c}")
            nc.sync.dma_start(out=t[:, :], in_=imgT[ds(c * P, P), :])
            img_sb.append(t)
            t = const.tile([P, E3], F16, tag=f"wq{c}")
            nc.sync.dma_start(out=t[:, :], in_=WqkvT[ds(c * P, P), :])
            wq_sb.append(t)
        for c in range(DC):
            t = const.tile([P, D], BF16, tag=f"wf{c}")
            nc.sync.dma_start(out=t[:, :], in_=WfcT[ds(c * P, P), :])
            wf_sb.append(t)

        bias_sb = const.tile([P, D], F32, tag="bias")
        b_ap = b_fc[:]
        b_bcast = bass.AP(tensor=b_ap.tensor, offset=b_ap.offset,
                          ap=[[0, P]] + list(b_ap.ap))
        nc.sync.dma_start(out=bias_sb[:, :], in_=b_bcast)

        ident = const.tile([P, P], F32, tag="ident")
        make_identity(nc, ident[:, :])
        ones64 = const.tile([1, DH], F16, tag="ones64")
        nc.vector.memset(ones64[:, :], 1.0)

        # V with ones column per head: [k-part, kt, h*65 + c], col 64 = 1
        vaug = const.tile([P, KT, H * 65], BF16, tag="vaug")
        nc.gpsimd.memset(vaug[:, :, :], 1.0)

        # merged attention output, transposed: [e in chunk, chunk, n]
        ot_sb = const.tile([P, DC, NT], BF16, tag="ot")

        # ---- V projection (natural layout), [128,512]+[128,256] halves ----
        def v_proj():
            for t in range(QT):
                psa = stp.tile([P, 512], F32, tag="st", name=f"psva{t}")
                psb = stp.tile([P, 512], F32, tag="st", name=f"psvb{t}")
                for c in range(DC):
                    lt = img_sb[c][:, ts(t, P)]
                    wv = wq_sb[c][:, :].rearrange("p (h x) -> p h x", h=H)
                    nc.tensor.matmul(psa[:, :].rearrange("p (h x) -> p h x", h=8),
                                     lt, wv[:, 0:8, 128:192],
                                     start=(c == 0), stop=(c == DC - 1))
                    nc.tensor.matmul(psb[:, 0:256].rearrange("p (h x) -> p h x", h=4),
                                     lt, wv[:, 8:12, 128:192],
                                     start=(c == 0), stop=(c == DC - 1))
                    yield
                vv = vaug[:, t, :].rearrange("p (h x) -> p h x", h=H)
                nc.scalar.copy(
                    vv[:, 0:8, 0:64],
                    psa[:, :].rearrange("p (h x) -> p h x", h=8))
                nc.scalar.copy(
                    vv[:, 8:12, 0:64],
                    psb[:, 0:256].rearrange("p (h x) -> p h x", h=4))
                yield

        # ---- qk projection for one head: psqk rows = [q(64); k(64)] ----
        def proj_head(h, sink):
            """Emit proj for head h; append (qa, ka) to sink when done."""
            psqk = pp.tile([P, NT], F32, tag="p", name=f"psqk{h}")
            lt = [wq_sb[c][:, ds(h * 192, P)] for c in range(DC)]
            for c in range(DC):
                for nb in range(2):
                    nc.tensor.matmul(psqk[:, ts(nb, 512)], lt[c],
                                     img_sb[c][:, ts(nb, 512)],
                                     start=(c == 0), stop=(c == DC - 1))
                yield
            qa = aug.tile([65, NT], F16, tag="qa", name=f"qa{h}")
            ka = aug.tile([65, NT], F16, tag="ka", name=f"ka{h}")
            nc.scalar.copy(qa[0:64, :], psqk[0:64, :])
            yield
            nc.scalar.copy(ka[0:64, :], psqk[64:128, :])
            nc.gpsimd.memset(ka[64:65, :], -1.0)
            sink[h] = (qa, ka)
            yield

        # ---- pass1 for one head: 8 S tiles + rowmax; then m-row via PE
        # transpose + DMA into qa row 64 ----
        def pass1_head(h, qk):
            qa, ka = qk[h]
            # half-maxima in [t][nb] order, combined to per-row maxima after
            mcat2 = small.tile([P, QT, 2], F32, tag="mcat2", name=f"mcat2_{h}")
            for t in range(QT):
                lt = qa[0:64, ts(t, P)]
                for nb in range(2):
                    psS = sp.tile([P, 512], F32, tag="s", name=f"psS{h}_{t}_{nb}")
                    nc.tensor.matmul(psS[:, :], lt,
                                     ka[0:64, ts(nb, 512)],
                                     start=True, stop=True)
                    nc.vector.reduce_max(out=mcat2[:, t, ds(nb, 1)],
                                         in_=psS[:, :],
                                         axis=mybir.AxisListType.X)
                    yield
            mcat = small.tile([P, QT], F32, tag="mcat", name=f"mcat{h}")
            nc.vector.reduce_max(out=mcat[:, :].rearrange("p (t o) -> p t o", o=1),
                                 in_=mcat2[:, :, :],
                                 axis=mybir.AxisListType.X)
            psT = sp.tile([8, P], F32, tag="s", name=f"psT{h}")
            nc.tensor.transpose(psT[:, :], mcat[:, :], ident[:, :])
            m_sb = small.tile([8, P], F16, tag="mrow", name=f"mrow{h}")
            nc.scalar.copy(m_sb[:, :], psT[:, :])
            nc.sync.dma_start(
                out=qa[ds(64, 1), :].rearrange("o (t x) -> o t x", t=QT),
                in_=m_sb[:, :])
            yield

        # ---- ST' + exp + PV stream for one head; then normalization ----
        # Pipelined at [128,512] grain. PV lags ST'/exp by LOOKAHEAD steps so
        # the first PV matmuls (which wait for the previous head's psOT
        # drain) never starve the PE FIFO of ready ST' work.
        LOOKAHEAD = 5
        def stpv_head(h, qk):
            qa, ka = qk[h]
            ot0 = op.tile([65, 512], F32, tag="o", name=f"psOT{h}a")
            ot1 = op.tile([65, 512], F32, tag="o", name=f"psOT{h}b")
            halves = (ot0, ot1)
            NI = 2 * KT
            pts = [None] * NI
            for i in range(NI + LOOKAHEAD):
                if i < NI:
                    kt, nb = i // 2, i % 2
                    psST = stp.tile([P, 512], F32, tag="st",
                                    name=f"psST{h}_{i}")
                    nc.tensor.matmul(psST[:, :], ka[:, ts(kt, P)],
                                     qa[:, ts(nb, 512)],
                                     start=True, stop=True)
                    pt_t = ptp.tile([P, 512], BF16, tag="pt",
                                    name=f"pt{h}_{i}")
                    nc.scalar.activation(pt_t[:, :], psST[:, :],
                                         mybir.ActivationFunctionType.Exp,
                                         bias=0.0, scale=8.0)
                    pts[i] = pt_t
                j = i - LOOKAHEAD
                if j >= 0:
                    kt, nb = j // 2, j % 2
                    nc.tensor.matmul(halves[nb][:, :],
                                     vaug[:, kt, ds(h * 65, 65)],
                                     pts[j][:, :],
                                     start=(kt == 0), stop=(kt == KT - 1))
                yield
            # normalization: rs = 1/s via exp(-ln s); broadcast via K=1
            # float32r matmul (full rate at 512 free)
            lns = small.tile([1, NT], F32, tag="lns", name=f"lns{h}")
            for nb in range(2):
                nc.scalar.activation(lns[:, ts(nb, 512)],
                                     halves[nb][ds(64, 1), :],
                                     mybir.ActivationFunctionType.Ln,
                                     bias=0.0, scale=1.0)
            rs_sb = small.tile([1, NT], F16, tag="rs", name=f"rs{h}")
            nc.scalar.activation(rs_sb[:, :], lns[:, :],
                                 mybir.ActivationFunctionType.Exp,
                                 bias=0.0, scale=-1.0)
            yield
            rs64_sb = outp.tile([DH, NT], F32, tag="rs64", name=f"rs64_{h}")
            for nb in range(2):
                psRS = stp.tile([DH, 512], F32, tag="st", name=f"psRS{h}_{nb}")
                nc.tensor.matmul(psRS[:, :], ones64[:, :],
                                 rs_sb[:, ts(nb, 512)],
                                 start=True, stop=True)
                nc.vector.tensor_copy(rs64_sb[:, ts(nb, 512)], psRS[:, :])
                yield
            for nb in range(2):
                nc.vector.tensor_mul(
                    ot_sb[ds((h % 2) * 64, 64), h // 2, ts(nb, 512)],
                    halves[nb][0:64, :], rs64_sb[:, ts(nb, 512)])
            yield

        # ---- fc + bias tail, [128,512]+[128,256] halves, 4 rotation slots ----
        def fc_tail():
            for t in range(QT):
                hp = (stp, sp)[t % 2]
                psa = hp.tile([P, 512], F32, tag=("st", "s")[t % 2],
                              name=f"psFa{t}")
                psb = hp.tile([P, 512], F32, tag=("st", "s")[t % 2],
                              name=f"psFb{t}")
                for c in range(DC):
                    nc.tensor.matmul(psa[:, :], ot_sb[:, c, ts(t, P)],
                                     wf_sb[c][:, 0:512],
                                     start=(c == 0), stop=(c == DC - 1))
                    nc.tensor.matmul(psb[:, 0:256], ot_sb[:, c, ts(t, P)],
                                     wf_sb[c][:, 512:768],
                                     start=(c == 0), stop=(c == DC - 1))
                    yield
                o_t = outp.tile([P, D], F32, tag="o", name=f"o{t}")
                nc.vector.tensor_add(o_t[:, 0:512], psa[:, :],
                                     bias_sb[:, 0:512])
                nc.vector.tensor_add(o_t[:, 512:768], psb[:, 0:256],
                                     bias_sb[:, 512:768])
                nc.sync.dma_start(out=out[ts(t, P), :], in_=o_t[:, :])
                yield

        # ---- schedule (repeat for in-NEFF throughput benchmarking) ----
        from itertools import chain as _chain
        for _rep in range(repeat):
            qk = {}
            _drive(v_proj(),
                   _chain(proj_head(0, qk), pass1_head(0, qk)),
                   _delay(_chain(proj_head(1, qk), proj_head(2, qk)), 16))
            # steady: head h's ST'/PV stream || pass1(h+1) || proj(h+3)
            for h in range(H):
                _drive(
                    stpv_head(h, qk),
                    pass1_head(h + 1, qk) if h + 1 < H else None,
                    proj_head(h + 3, qk) if h + 3 < H else None,
                )
            _drive(fc_tail())

    _split_sync_waits(nc)
    return nc


_NC_CACHE = {}


def _get_graph(repeat=1):
    if repeat not in _NC_CACHE:
        _NC_CACHE[repeat] = build_graph(repeat)
    return _NC_CACHE[repeat]


def _install_compile_memo():
    import hashlib
    import shutil
    from concourse import bass_utils as bu
    from concourse import bass2jax
    if getattr(bu.compile_bir_kernel, "_memo", False):
        return
    orig = bu.compile_bir_kernel

    def memo_compile(bir_json, tmpdir, neff_name="file.neff"):
        key = hashlib.sha256(bir_json).hexdigest()
        os.makedirs("/tmp/neff_cache", exist_ok=True)
        persist = f"/tmp/neff_cache/{key}.neff"
        if os.path.exists(persist):
            return persist
        r = orig(bir_json, tmpdir, neff_name)
        shutil.copyfile(r, persist)
        return persist
    memo_compile._memo = True
    bu.compile_bir_kernel = memo_compile
    bass2jax.compile_bir_kernel = memo_compile


_EXEC_CACHE = {}


def _get_executor(repeat=1):
    _install_compile_memo()
    if repeat in _EXEC_CACHE:
        return _EXEC_CACHE[repeat]
    import jax
    from jax.sharding import Mesh, PartitionSpec
    from jax.experimental.shard_map import shard_map
    from concourse import mybir as mb
    from concourse import bass2jax

    bass2jax.install_neuronx_cc_hook()
    nc = _get_graph(repeat)
    partition_name = (nc.partition_id_tensor.name
                      if nc.partition_id_tensor else None)
    in_names, out_names, out_avals, zero_outs = [], [], [], []
    for alloc in nc.m.functions[0].allocations:
        if not isinstance(alloc, mb.MemoryLocationSet):
            continue
        name = alloc.memorylocations[0].name
        if alloc.kind == "ExternalInput":
            if name != partition_name:
                in_names.append(name)
        elif alloc.kind == "ExternalOutput":
            shape = tuple(alloc.tensor_shape)
            dtype = mb.dt.np(alloc.dtype)
            out_names.append(name)
            out_avals.append(jax.core.ShapedArray(shape, dtype))
            zero_outs.append(np.zeros(shape, dtype))
    n_params = len(in_names)
    all_in_names = list(in_names) + list(out_names)
    if partition_name is not None:
        all_in_names.append(partition_name)

    def _body(*args):
        operands = list(args)
        if partition_name is not None:
            operands.append(bass2jax.partition_id_tensor())
        outs = bass2jax._bass_exec_p.bind(
            *operands,
            out_avals=tuple(out_avals),
            in_names=tuple(all_in_names),
            out_names=tuple(out_names),
            lowering_input_output_aliases=(),
            sim_require_finite=True,
            sim_require_nnan=True,
            nc=nc,
        )
        return tuple(outs)

    devices = jax.devices()[:N_CORES]
    mesh = Mesh(np.asarray(devices), ("core",))
    n_outs = len(out_names)
    in_specs = (PartitionSpec("core"),) * (n_params + n_outs)
    out_specs = (PartitionSpec("core"),) * n_outs
    sharded = jax.jit(shard_map(_body, mesh=mesh, in_specs=in_specs,
                                out_specs=out_specs, check_rep=False))
    ex = dict(fn=sharded, in_names=in_names, out_names=out_names,
              out_avals=out_avals, zero_outs=zero_outs, n_params=n_params)
    _EXEC_CACHE[repeat] = ex
    return ex


def _prep_inputs(img, W_qkv, W_fc, b_fc):
    img = np.asarray(img, dtype=np.float32)
    W_qkv = np.asarray(W_qkv, dtype=np.float32)
    W_fc = np.asarray(W_fc, dtype=np.float32)
    b_fc = np.asarray(b_fc, dtype=np.float32)
    imgT = np.ascontiguousarray(img.transpose(0, 2, 1)).astype(np.float16)
    WqkvT = np.ascontiguousarray(W_qkv.T).astype(np.float16)
    WfcT = np.ascontiguousarray(W_fc.T).astype(ml_dtypes.bfloat16)
    return [{"imgT": imgT[i], "WqkvT": WqkvT, "WfcT": WfcT, "b_fc": b_fc}
            for i in range(N_CORES)]


def _run_cached(in_maps):
    ex = _get_executor()
    n_cores = N_CORES
    per_core = [[np.asarray(m[name]) for name in ex["in_names"]]
                for m in in_maps]
    concat_in = [np.concatenate([per_core[c][i] for c in range(n_cores)], axis=0)
                 for i in range(ex["n_params"])]
    concat_zeros = [np.zeros((n_cores * z.shape[0], *z.shape[1:]), z.dtype)
                    for z in ex["zero_outs"]]
    out_arrs = ex["fn"](*concat_in, *concat_zeros)
    outs = [
        {name: np.asarray(out_arrs[i]).reshape(n_cores, *ex["out_avals"][i].shape)[c]
         for i, name in enumerate(ex["out_names"])}
        for c in range(n_cores)
    ]
    return outs


def bench_resident(m1=10, m2=40):
    """Per-call cost with device-resident inputs and a single executable:
    slope between m1 and m2 sequential async dispatches."""
    import time
    import jax
    from jax.sharding import Mesh, PartitionSpec, NamedSharding
    ex = _get_executor()
    z = np.load("/root/problem/_expected.npz")
    in_maps = _prep_inputs(z["img"], z["W_qkv"], z["W_fc"], z["b_fc"])
    per_core = [[np.asarray(m[k]) for k in ex["in_names"]] for m in in_maps]
    concat_in = [np.concatenate([per_core[c][i] for c in range(N_CORES)], axis=0)
                 for i in range(ex["n_params"])]
    concat_zeros = [np.zeros((N_CORES * z_.shape[0], *z_.shape[1:]), z_.dtype)
                    for z_ in ex["zero_outs"]]
    devices = jax.devices()[:N_CORES]
    mesh = Mesh(np.asarray(devices), ("core",))
    sh = NamedSharding(mesh, PartitionSpec("core"))
    dev_in = [jax.device_put(a, sh) for a in concat_in]
    dev_zero = [jax.device_put(a, sh) for a in concat_zeros]
    jax.block_until_ready(dev_in + dev_zero)
    fn = ex["fn"]
    o = fn(*dev_in, *dev_zero)
    jax.block_until_ready(o)
    res = {}
    for m in (m1, m2):
        best = None
        for _ in range(3):
            t0 = time.perf_counter()
            outs = [fn(*dev_in, *dev_zero) for _ in range(m)]
            jax.block_until_ready(outs)
            dt = time.perf_counter() - t0
            best = dt if best is None else min(best, dt)
        res[m] = best
        print(f"m={m}: {best*1e3:.2f} ms total, {best/m*1e3:.3f} ms/call")
    slope = (res[m2] - res[m1]) / (m2 - m1)
    print(f"slope (per-call device cost) ~= {slope*1e6:.1f} us")
    return slope


def _run(img, W_qkv, W_fc, b_fc, trace=False, tmpdir=None):
    in_maps = _prep_inputs(img, W_qkv, W_fc, b_fc)
    results = _run_cached(in_maps)
    outs = np.stack([np.asarray(results[i]["out"], dtype=np.float32)
                     for i in range(N_CORES)])
    return outs, None


def kernel(img, W_qkv, W_fc, b_fc):
    outs, _ = _run(img, W_qkv, W_fc, b_fc)
    return outs
